# revision 16
# baseline (speedup 1.0000x reference)
"""AdaptiveGridMerger Trainium2 kernel.

Math: the reference scatters x[b,c,:] into a flat 8x8 grid with bilinear
(4-corner) weights from positions[b,c,:], then matmuls grid_weights.
Equivalent form used here: out[b] = GW @ (S[b] @ x[b]) where
S[b] in R[64,306] holds channel c's 4 corner weights in column c.
S[b].T (layout [c, g]) is built on-device: floor/weights on the vector
engine, then one broadcast outer-product wy (x) wx per row (the 8x8
grid factorizes). Both contractions run on the TensorEngine in bf16.

Sharding: data-parallel over batch, 2 batches per core, grid_weights
replicated (pre-transposed on host to [64, 270] for the lhsT layout).

Perf structure: spin matmuls pre-ramp the PE clock (HAM gate) during
setup; weights-outer matmul ordering overlaps drain/fill; 2-bank PSUM
tiles let PSUM->SBUF copies run 1024 wide, split across DVE and ACT so
the PE never stalls on PSUM reuse.
"""

import numpy as np

import concourse.bass as bass
import concourse.bacc as bacc
import concourse.mybir as mybir
from concourse import tile
from concourse.bass_utils import run_bass_kernel_spmd

B, C, T = 16, 306, 4096
M, G, GS = 270, 64, 8
N_CORES = 8
BL = B // N_CORES  # batches per core

C_CHUNKS = [(0, 128), (128, 128), (256, 50)]
M_CHUNKS = [(0, 128), (128, 128), (256, 14)]
T_DMA = 2048
T_PS = 512
NPS = T_DMA // T_PS
N_SPIN = 12

MM_DTYPE = mybir.dt.bfloat16
NP_MM = mybir.dt.np(MM_DTYPE)

FP32 = mybir.dt.float32
OP = mybir.AluOpType


def build_nc():
    nc = bacc.Bacc()
    x_ext = nc.declare_dram_parameter("x", [BL, C, T], MM_DTYPE, isOutput=False)
    pos_ext = nc.declare_dram_parameter("positions", [BL, C, 2], FP32, isOutput=False)
    gwt_ext = nc.declare_dram_parameter("gw_t", [G, M], MM_DTYPE, isOutput=False)
    out_ext = nc.declare_dram_parameter("out", [BL, M, T], MM_DTYPE, isOutput=True)

    n_chunks = len(C_CHUNKS)
    with tile.TileContext(nc) as tc:
        with (
            tc.tile_pool(name="const", bufs=1) as constp,
            tc.tile_pool(name="stp", bufs=1) as stp,
            tc.tile_pool(name="scr", bufs=1) as scr,
            tc.tile_pool(name="xp", bufs=3) as xp,
            tc.tile_pool(name="op", bufs=2) as outp,
            tc.tile_pool(name="gvp", bufs=3) as gvp,
            tc.tile_pool(name="ps_gv", bufs=1, space=bass.MemorySpace.PSUM) as ps_gv,
            tc.tile_pool(name="ps_out", bufs=2, space=bass.MemorySpace.PSUM) as ps_out,
        ):
            # ---- PE clock pre-ramp: spin matmuls on a zero tile while the
            # ---- DMAs and the DVE setup below run. HAM releases the 2.4 GHz
            # ---- clock only after ~5 us of sustained PE activity.
            dummy = constp.tile([128, T_PS], MM_DTYPE, tag="dummy")
            nc.gpsimd.memset(dummy[:], 0.0)
            spin_ps = ps_out.tile([128, 2 * T_PS], FP32, tag="o_ps", name="spin_ps")
            for s in range(N_SPIN):
                nc.tensor.matmul(
                    spin_ps[:, :T_PS], dummy[:, :128], dummy[:], start=True, stop=True
                )

            gw_t = constp.tile([G, M], MM_DTYPE, tag="gw_t")
            nc.sync.dma_start(out=gw_t[:], in_=gwt_ext[:])

            # iota rows [0..7] and [-1..6]: (iota0 == low) selects the low
            # cell, (iotam1 == low) the low+1 cell.
            iot = {}
            for nm, base in (("i0", 0), ("im1", -1)):
                tg = constp.tile([128, GS], FP32, tag=f"iog{nm}", name=f"iog{nm}")
                nc.gpsimd.iota(
                    tg[:],
                    pattern=[[1, GS]],
                    base=base,
                    channel_multiplier=0,
                    allow_small_or_imprecise_dtypes=True,
                )
                tv = constp.tile([128, GS], FP32, tag=f"iov{nm}", name=f"iov{nm}")
                nc.vector.tensor_copy(tv[:], tg[:])
                iot[nm] = tv

            # ---- ST build. All 6 (batch, chunk) column-pairs share one
            # ---- [128, 12] tile so the floor/weight math is 7 wide ops.
            NCOL = 2 * BL * n_chunks
            pos_all = scr.tile([128, NCOL], FP32, tag="pos_all")
            nc.vector.memset(pos_all[:], 0.0)
            for b in range(BL):
                for ci, (c0, cn) in enumerate(C_CHUNKS):
                    col = 2 * (b * n_chunks + ci)
                    nc.sync.dma_start(
                        out=pos_all[:cn, col : col + 2],
                        in_=pos_ext[b, c0 : c0 + cn, :],
                    )
            # grid_pos = (pos + 1) * 4, exact vs reference's *8/2
            gp = scr.tile([128, NCOL], FP32, tag="gp")
            nc.vector.tensor_scalar(gp[:], pos_all[:], 1.0, GS / 2.0, OP.add, OP.mult)
            # floor(): int cast, then subtract 1 where the cast rounded up
            ilow = scr.tile([128, NCOL], mybir.dt.int32, tag="ilow")
            nc.vector.tensor_copy(ilow[:], gp[:])
            flow = scr.tile([128, NCOL], FP32, tag="flow")
            nc.vector.tensor_copy(flow[:], ilow[:])
            mask = scr.tile([128, NCOL], FP32, tag="mask")
            nc.vector.tensor_tensor(mask[:], flow[:], gp[:], OP.is_gt)
            low = scr.tile([128, NCOL], FP32, tag="low")
            nc.vector.tensor_tensor(low[:], flow[:], mask[:], OP.subtract)
            whi = scr.tile([128, NCOL], FP32, tag="whi")
            nc.vector.tensor_tensor(whi[:], gp[:], low[:], OP.subtract)
            wlo = scr.tile([128, NCOL], FP32, tag="wlo")
            nc.vector.tensor_scalar(wlo[:], whi[:], -1.0, 1.0, OP.mult, OP.add)

            st_tiles = {}
            for b in range(BL):
                for ci, (c0, cn) in enumerate(C_CHUNKS):
                    sfx = f"{b}_{ci}"
                    col = 2 * (b * n_chunks + ci)
                    wyx = []
                    for d, nm in ((0, "wy"), (1, "wx")):
                        cd = col + d
                        t1 = scr.tile([128, GS], FP32, tag=f"{nm}a{sfx}", name=f"{nm}a{sfx}")
                        nc.vector.tensor_scalar(
                            t1[:cn], iot["i0"][:cn], low[:cn, cd : cd + 1],
                            wlo[:cn, cd : cd + 1], OP.is_equal, OP.mult,
                        )
                        t2 = scr.tile([128, GS], FP32, tag=f"{nm}b{sfx}", name=f"{nm}b{sfx}")
                        nc.vector.tensor_scalar(
                            t2[:cn], iot["im1"][:cn], low[:cn, cd : cd + 1],
                            whi[:cn, cd : cd + 1], OP.is_equal, OP.mult,
                        )
                        tw = scr.tile([128, GS], FP32, tag=f"{nm}{sfx}", name=f"{nm}{sfx}")
                        nc.vector.tensor_tensor(tw[:cn], t1[:cn], t2[:cn], OP.add)
                        wyx.append(tw)
                    st = stp.tile([128, G], MM_DTYPE, tag=f"st{sfx}", name=f"st{sfx}")
                    nc.vector.tensor_tensor(
                        st[:cn].rearrange("c (i j) -> c i j", i=GS),
                        wyx[0][:cn].unsqueeze(2).broadcast_to((cn, GS, GS)),
                        wyx[1][:cn].unsqueeze(1).broadcast_to((cn, GS, GS)),
                        OP.mult,
                    )
                    st_tiles[(b, ci)] = st

            # Warm-up matmuls: PE observes the DVE/DMA semaphores here so
            # steady-state matmuls carry few pending waits.
            warm = ps_out.tile([128, 2 * T_PS], FP32, tag="o_ps", name="warm")
            for b in range(BL):
                for ci, (c0, cn) in enumerate(C_CHUNKS):
                    st = st_tiles[(b, ci)]
                    nc.tensor.matmul(
                        warm[:G, :G], st[:cn], st[:cn, :G], start=True, stop=True
                    )
            nc.tensor.matmul(
                warm[:, :G], gw_t[:, :128], gw_t[:, :G], start=True, stop=True
            )

            # ---- Main loop: gv = ST.T @ x, out = GW @ gv, weights-outer.
            for b in range(BL):
                for tt in range(T // T_DMA):
                    t0 = tt * T_DMA
                    xts = []
                    for ci, (c0, cn) in enumerate(C_CHUNKS):
                        xt = xp.tile([128, T_DMA], MM_DTYPE, tag=f"x{ci}", name=f"x{ci}")
                        nc.sync.dma_start(
                            out=xt[:cn], in_=x_ext[b, c0 : c0 + cn, t0 : t0 + T_DMA]
                        )
                        xts.append(xt)
                    outs = []
                    for mi, (m0, mn) in enumerate(M_CHUNKS):
                        outs.append(
                            outp.tile([128, T_DMA], MM_DTYPE, tag=f"o{mi}", name=f"o{mi}")
                        )

                    # h-blocked: mm1 for half h, ACT copies gv while the PE
                    # moves on to the next half / mm2 -- no PE bubble.
                    gv_sbs = []
                    for h in range(NPS // 2):
                        gv_ps = ps_gv.tile(
                            [G, 2 * T_PS], FP32, tag=f"gv{h}", name=f"gv{h}"
                        )
                        for ci, (c0, cn) in enumerate(C_CHUNKS):
                            for q in range(2):
                                f0 = (h * 2 + q) * T_PS
                                nc.tensor.matmul(
                                    gv_ps[:, q * T_PS : (q + 1) * T_PS],
                                    st_tiles[(b, ci)][:cn],
                                    xts[ci][:cn, f0 : f0 + T_PS],
                                    start=(ci == 0),
                                    stop=(ci == n_chunks - 1),
                                    skip_group_check=True,
                                )
                        gv_sb = gvp.tile(
                            [G, 2 * T_PS], MM_DTYPE, tag=f"gv_sb{h}", name=f"gv_sb{h}"
                        )
                        nc.scalar.copy(gv_sb[:], gv_ps[:])
                        gv_sbs.append(gv_sb)

                    # mm2: h-outer so each half starts as soon as its gv lands
                    for h in range(NPS // 2):
                        for mi, (m0, mn) in enumerate(M_CHUNKS):
                            o_ps = ps_out.tile(
                                [128, 2 * T_PS], FP32, tag="o_ps", name="o_ps"
                            )
                            for q in range(2):
                                nc.tensor.matmul(
                                    o_ps[:mn, q * T_PS : (q + 1) * T_PS],
                                    gw_t[:, m0 : m0 + mn],
                                    gv_sbs[h][:, q * T_PS : (q + 1) * T_PS],
                                    start=True,
                                    stop=True,
                                )
                            f0 = h * 2 * T_PS
                            dst = outs[mi][:mn, f0 : f0 + 2 * T_PS]
                            if mi == 2:
                                nc.scalar.copy(dst, o_ps[:mn])
                            else:
                                nc.vector.tensor_copy(dst, o_ps[:mn])
                    for mi, (m0, mn) in enumerate(M_CHUNKS):
                        nc.sync.dma_start(
                            out=out_ext[b, m0 : m0 + mn, t0 : t0 + T_DMA],
                            in_=outs[mi][:mn],
                        )
    nc.compile()
    return nc


def make_in_maps(x, positions, grid_weights):
    gw_t = np.ascontiguousarray(grid_weights.T).astype(NP_MM)
    in_maps = []
    for i in range(N_CORES):
        sl = slice(i * BL, (i + 1) * BL)
        in_maps.append(
            {
                "x": np.ascontiguousarray(x[sl]).astype(NP_MM),
                "positions": np.ascontiguousarray(positions[sl]),
                "gw_t": gw_t,
            }
        )
    return in_maps


_NC_CACHE = None


def kernel(x, positions, grid_weights):
    global _NC_CACHE
    if _NC_CACHE is None:
        _NC_CACHE = build_nc()
    nc = _NC_CACHE
    in_maps = make_in_maps(x, positions, grid_weights)
    res = run_bass_kernel_spmd(nc, in_maps, core_ids=list(range(N_CORES)))
    out = np.concatenate([r["out"] for r in res.results], axis=0)
    return np.asarray(out, dtype=np.float32)


if __name__ == "__main__":
    xs = np.random.randn(B, C, T).astype(np.float32)
    ps = np.random.uniform(-1, 0.74, (B, C, 2)).astype(np.float32)
    gw = np.random.randn(M, G).astype(np.float32)
    out = kernel(xs, ps, gw)
    print(out.shape, out.dtype)


# revision 17
# speedup vs baseline: 1.1757x; 1.1757x over previous
"""AdaptiveGridMerger Trainium2 kernel.

Math: the reference scatters x[b,c,:] into a flat 8x8 grid with bilinear
(4-corner) weights from positions[b,c,:], then matmuls grid_weights.
Equivalent form used here: out[b] = GW @ (S[b] @ x[b]) where
S[b] in R[64,306] holds channel c's 4 corner weights in column c.
S[b].T (layout [c, g]) is built on-device: floor/weights on the vector
engine, then one broadcast outer-product wy (x) wx per row (the 8x8
grid factorizes). Both contractions run on the TensorEngine in bf16.

Sharding: data-parallel over batch, 2 batches per core, grid_weights
replicated (pre-transposed on host to [64, 270] for the lhsT layout).

Perf structure: spin matmuls pre-ramp the PE clock (HAM gate) during
setup; weights-outer matmul ordering overlaps drain/fill; 2-bank PSUM
tiles let PSUM->SBUF copies run 1024 wide, split across DVE and ACT so
the PE never stalls on PSUM reuse.
"""

import numpy as np

import concourse.bass as bass
import concourse.bacc as bacc
import concourse.mybir as mybir
from concourse import tile
from concourse.bass_utils import run_bass_kernel_spmd

B, C, T = 16, 306, 4096
M, G, GS = 270, 64, 8
N_CORES = 8
BL = B // N_CORES  # batches per core

C_CHUNKS = [(0, 128), (128, 128), (256, 50)]
M_CHUNKS = [(0, 128), (128, 128), (256, 14)]
T_DMA = 2048
T_PS = 512
NPS = T_DMA // T_PS
N_SPIN = 14

MM_DTYPE = mybir.dt.bfloat16
NP_MM = mybir.dt.np(MM_DTYPE)

FP32 = mybir.dt.float32
OP = mybir.AluOpType


def build_nc():
    nc = bacc.Bacc()
    x_ext = nc.declare_dram_parameter("x", [BL, C, T], MM_DTYPE, isOutput=False)
    pos_ext = nc.declare_dram_parameter("positions", [BL, C, 2], FP32, isOutput=False)
    gwt_ext = nc.declare_dram_parameter("gw_t", [G, M], MM_DTYPE, isOutput=False)
    out_ext = nc.declare_dram_parameter("out", [BL, M, T], MM_DTYPE, isOutput=True)

    n_chunks = len(C_CHUNKS)
    with tile.TileContext(nc) as tc:
        with (
            tc.tile_pool(name="const", bufs=1) as constp,
            tc.tile_pool(name="stp", bufs=1) as stp,
            tc.tile_pool(name="scr", bufs=1) as scr,
            tc.tile_pool(name="xp", bufs=3) as xp,
            tc.tile_pool(name="op", bufs=2) as outp,
            tc.tile_pool(name="gvp", bufs=3) as gvp,
            tc.tile_pool(name="ps_gv", bufs=1, space=bass.MemorySpace.PSUM) as ps_gv,
            tc.tile_pool(name="ps_out", bufs=2, space=bass.MemorySpace.PSUM) as ps_out,
        ):
            # ---- PE clock pre-ramp: spin matmuls on a zero tile while the
            # ---- DMAs and the DVE setup below run. HAM releases the 2.4 GHz
            # ---- clock only after ~5 us of sustained PE activity.
            dummy = constp.tile([128, T_PS], MM_DTYPE, tag="dummy")
            nc.gpsimd.memset(dummy[:], 0.0)
            spin_ps = ps_out.tile([128, 2 * T_PS], FP32, tag="o_ps", name="spin_ps")
            for s in range(N_SPIN):
                nc.tensor.matmul(
                    spin_ps[:, :T_PS], dummy[:, :128], dummy[:], start=True, stop=True
                )

            gw_t = constp.tile([G, M], MM_DTYPE, tag="gw_t")
            nc.sync.dma_start(out=gw_t[:], in_=gwt_ext[:])

            # iota rows [0..7] and [-1..6]: (iota0 == low) selects the low
            # cell, (iotam1 == low) the low+1 cell.
            iot = {}
            for nm, base in (("i0", 0), ("im1", -1)):
                tg = constp.tile([128, GS], FP32, tag=f"iog{nm}", name=f"iog{nm}")
                nc.gpsimd.iota(
                    tg[:],
                    pattern=[[1, GS]],
                    base=base,
                    channel_multiplier=0,
                    allow_small_or_imprecise_dtypes=True,
                )
                tv = constp.tile([128, GS], FP32, tag=f"iov{nm}", name=f"iov{nm}")
                nc.vector.tensor_copy(tv[:], tg[:])
                iot[nm] = tv

            # ---- ST build. All 6 (batch, chunk) column-pairs share one
            # ---- [128, 12] tile so the floor/weight math is 7 wide ops.
            NCOL = 2 * BL * n_chunks
            pos_all = scr.tile([128, NCOL], FP32, tag="pos_all")
            nc.vector.memset(pos_all[:], 0.0)
            for b in range(BL):
                for ci, (c0, cn) in enumerate(C_CHUNKS):
                    col = 2 * (b * n_chunks + ci)
                    nc.sync.dma_start(
                        out=pos_all[:cn, col : col + 2],
                        in_=pos_ext[b, c0 : c0 + cn, :],
                    )
            # grid_pos = (pos + 1) * 4, exact vs reference's *8/2
            gp = scr.tile([128, NCOL], FP32, tag="gp")
            nc.vector.tensor_scalar(gp[:], pos_all[:], 1.0, GS / 2.0, OP.add, OP.mult)
            # floor(): int cast, then subtract 1 where the cast rounded up
            ilow = scr.tile([128, NCOL], mybir.dt.int32, tag="ilow")
            nc.vector.tensor_copy(ilow[:], gp[:])
            flow = scr.tile([128, NCOL], FP32, tag="flow")
            nc.vector.tensor_copy(flow[:], ilow[:])
            mask = scr.tile([128, NCOL], FP32, tag="mask")
            nc.vector.tensor_tensor(mask[:], flow[:], gp[:], OP.is_gt)
            low = scr.tile([128, NCOL], FP32, tag="low")
            nc.vector.tensor_tensor(low[:], flow[:], mask[:], OP.subtract)
            whi = scr.tile([128, NCOL], FP32, tag="whi")
            nc.vector.tensor_tensor(whi[:], gp[:], low[:], OP.subtract)
            wlo = scr.tile([128, NCOL], FP32, tag="wlo")
            nc.vector.tensor_scalar(wlo[:], whi[:], -1.0, 1.0, OP.mult, OP.add)

            st_tiles = {}
            for b in range(BL):
                for ci, (c0, cn) in enumerate(C_CHUNKS):
                    sfx = f"{b}_{ci}"
                    col = 2 * (b * n_chunks + ci)
                    wyx = []
                    for d, nm in ((0, "wy"), (1, "wx")):
                        cd = col + d
                        t1 = scr.tile([128, GS], FP32, tag=f"{nm}a{sfx}", name=f"{nm}a{sfx}")
                        nc.vector.tensor_scalar(
                            t1[:cn], iot["i0"][:cn], low[:cn, cd : cd + 1],
                            wlo[:cn, cd : cd + 1], OP.is_equal, OP.mult,
                        )
                        t2 = scr.tile([128, GS], FP32, tag=f"{nm}b{sfx}", name=f"{nm}b{sfx}")
                        nc.vector.tensor_scalar(
                            t2[:cn], iot["im1"][:cn], low[:cn, cd : cd + 1],
                            whi[:cn, cd : cd + 1], OP.is_equal, OP.mult,
                        )
                        tw = scr.tile([128, GS], FP32, tag=f"{nm}{sfx}", name=f"{nm}{sfx}")
                        nc.vector.tensor_tensor(tw[:cn], t1[:cn], t2[:cn], OP.add)
                        wyx.append(tw)
                    st = stp.tile([128, G], MM_DTYPE, tag=f"st{sfx}", name=f"st{sfx}")
                    nc.vector.tensor_tensor(
                        st[:cn].rearrange("c (i j) -> c i j", i=GS),
                        wyx[0][:cn].unsqueeze(2).broadcast_to((cn, GS, GS)),
                        wyx[1][:cn].unsqueeze(1).broadcast_to((cn, GS, GS)),
                        OP.mult,
                    )
                    st_tiles[(b, ci)] = st

            # Warm-up matmuls: PE observes the DVE/DMA semaphores here so
            # steady-state matmuls carry few pending waits.
            warm = ps_out.tile([128, 2 * T_PS], FP32, tag="o_ps", name="warm")
            for b in range(BL):
                for ci, (c0, cn) in enumerate(C_CHUNKS):
                    st = st_tiles[(b, ci)]
                    nc.tensor.matmul(
                        warm[:G, :G], st[:cn], st[:cn, :G], start=True, stop=True
                    )
            nc.tensor.matmul(
                warm[:, :G], gw_t[:, :128], gw_t[:, :G], start=True, stop=True
            )

            # ---- Main loop: gv = ST.T @ x, out = GW @ gv, weights-outer.
            for b in range(BL):
                for tt in range(T // T_DMA):
                    t0 = tt * T_DMA
                    xts = []
                    for ci, (c0, cn) in enumerate(C_CHUNKS):
                        xt = xp.tile([128, T_DMA], MM_DTYPE, tag=f"x{ci}", name=f"x{ci}")
                        nc.sync.dma_start(
                            out=xt[:cn], in_=x_ext[b, c0 : c0 + cn, t0 : t0 + T_DMA]
                        )
                        xts.append(xt)
                    outs = []
                    for mi, (m0, mn) in enumerate(M_CHUNKS):
                        outs.append(
                            outp.tile([128, T_DMA], MM_DTYPE, tag=f"o{mi}", name=f"o{mi}")
                        )

                    # h-blocked: mm1 for half h, ACT copies gv while the PE
                    # moves on to the next half / mm2 -- no PE bubble.
                    gv_sbs = []
                    for h in range(NPS // 2):
                        gv_ps = ps_gv.tile(
                            [G, 2 * T_PS], FP32, tag=f"gv{h}", name=f"gv{h}"
                        )
                        for ci, (c0, cn) in enumerate(C_CHUNKS):
                            for q in range(2):
                                f0 = (h * 2 + q) * T_PS
                                nc.tensor.matmul(
                                    gv_ps[:, q * T_PS : (q + 1) * T_PS],
                                    st_tiles[(b, ci)][:cn],
                                    xts[ci][:cn, f0 : f0 + T_PS],
                                    start=(ci == 0),
                                    stop=(ci == n_chunks - 1),
                                    skip_group_check=True,
                                )
                        gv_sb = gvp.tile(
                            [G, 2 * T_PS], MM_DTYPE, tag=f"gv_sb{h}", name=f"gv_sb{h}"
                        )
                        nc.scalar.copy(gv_sb[:], gv_ps[:])
                        gv_sbs.append(gv_sb)

                    # mm2: h-outer so each half starts as soon as its gv lands
                    for h in range(NPS // 2):
                        for mi, (m0, mn) in enumerate(M_CHUNKS):
                            o_ps = ps_out.tile(
                                [128, 2 * T_PS], FP32, tag="o_ps", name="o_ps"
                            )
                            for q in range(2):
                                nc.tensor.matmul(
                                    o_ps[:mn, q * T_PS : (q + 1) * T_PS],
                                    gw_t[:, m0 : m0 + mn],
                                    gv_sbs[h][:, q * T_PS : (q + 1) * T_PS],
                                    start=True,
                                    stop=True,
                                )
                            f0 = h * 2 * T_PS
                            dst = outs[mi][:mn, f0 : f0 + 2 * T_PS]
                            if mi == 2:
                                nc.scalar.copy(dst, o_ps[:mn])
                            else:
                                nc.vector.tensor_copy(dst, o_ps[:mn])
                    for mi, (m0, mn) in enumerate(M_CHUNKS):
                        nc.sync.dma_start(
                            out=out_ext[b, m0 : m0 + mn, t0 : t0 + T_DMA],
                            in_=outs[mi][:mn],
                        )
    nc.compile()
    return nc


def make_in_maps(x, positions, grid_weights):
    gw_t = np.ascontiguousarray(grid_weights.T).astype(NP_MM)
    in_maps = []
    for i in range(N_CORES):
        sl = slice(i * BL, (i + 1) * BL)
        in_maps.append(
            {
                "x": np.ascontiguousarray(x[sl]).astype(NP_MM),
                "positions": np.ascontiguousarray(positions[sl]),
                "gw_t": gw_t,
            }
        )
    return in_maps


_NC_CACHE = None


def kernel(x, positions, grid_weights):
    global _NC_CACHE
    if _NC_CACHE is None:
        _NC_CACHE = build_nc()
    nc = _NC_CACHE
    in_maps = make_in_maps(x, positions, grid_weights)
    res = run_bass_kernel_spmd(nc, in_maps, core_ids=list(range(N_CORES)))
    out = np.concatenate([r["out"] for r in res.results], axis=0)
    return np.asarray(out, dtype=np.float32)


if __name__ == "__main__":
    xs = np.random.randn(B, C, T).astype(np.float32)
    ps = np.random.uniform(-1, 0.74, (B, C, 2)).astype(np.float32)
    gw = np.random.randn(M, G).astype(np.float32)
    out = kernel(xs, ps, gw)
    print(out.shape, out.dtype)


# revision 21
# speedup vs baseline: 1.2839x; 1.0920x over previous
"""AdaptiveGridMerger Trainium2 kernel.

Math: the reference scatters x[b,c,:] into a flat 8x8 grid with bilinear
(4-corner) weights from positions[b,c,:], then matmuls grid_weights.
Equivalent form used here: out[b] = GW @ (S[b] @ x[b]) where
S[b] in R[64,306] holds channel c's 4 corner weights in column c.
Row c of S[b].T factorizes as wy (x) wx with wy[j] = relu(1-|gp0-j|)
(the bilinear hat function; exact vs the reference's floor/frac math by
Sterbenz). Both contractions run on the TensorEngine in bf16.

Sharding: data-parallel over batch, 2 batches per core, grid_weights
replicated (pre-transposed on host to [64, 270] for the lhsT layout).

Perf structure: spin matmuls pre-ramp the PE clock (HAM gate) during
setup; weights-outer matmul ordering overlaps drain/fill; input DMAs
issue from GpSimd so output DMAs can't head-of-line block them; PSUM ->
SBUF copies are split across DVE and ACT so the PE never stalls.
"""

import numpy as np

import concourse.bass as bass
import concourse.bacc as bacc
import concourse.mybir as mybir
from concourse import tile
from concourse.bass_utils import run_bass_kernel_spmd

B, C, T = 16, 306, 4096
M, G, GS = 270, 64, 8
N_CORES = 8
BL = B // N_CORES  # batches per core

C_CHUNKS = [(0, 128), (128, 128), (256, 50)]
M_CHUNKS = [(0, 128), (128, 128), (256, 14)]
T_DMA = 2048
T_PS = 512
NPS = T_DMA // T_PS
N_SPIN = 16

MM_DTYPE = mybir.dt.bfloat16
NP_MM = mybir.dt.np(MM_DTYPE)

FP32 = mybir.dt.float32
OP = mybir.AluOpType


def _pos_col(b, ci):
    # column pair layout in pos_all/gp: full 128-row chunks first (one
    # rearranged DMA), the two 50-row tails last (second DMA)
    return 2 * (b * 2 + ci) if ci < 2 else 8 + 2 * b


def build_nc():
    nc = bacc.Bacc()
    x_ext = nc.declare_dram_parameter("x", [BL, C, T], MM_DTYPE, isOutput=False)
    pos_ext = nc.declare_dram_parameter("positions", [BL, C, 2], FP32, isOutput=False)
    gwt_ext = nc.declare_dram_parameter("gw_t", [G, M], MM_DTYPE, isOutput=False)
    out_ext = nc.declare_dram_parameter("out", [BL, M, T], MM_DTYPE, isOutput=True)

    n_chunks = len(C_CHUNKS)
    with tile.TileContext(nc) as tc:
        with (
            tc.tile_pool(name="const", bufs=1) as constp,
            tc.tile_pool(name="stp", bufs=1) as stp,
            tc.tile_pool(name="scr", bufs=1) as scr,
            tc.tile_pool(name="xp", bufs=3) as xp,
            tc.tile_pool(name="op", bufs=2) as outp,
            tc.tile_pool(name="gvp", bufs=3) as gvp,
            tc.tile_pool(name="ps_gv", bufs=1, space=bass.MemorySpace.PSUM) as ps_gv,
            tc.tile_pool(name="ps_out", bufs=2, space=bass.MemorySpace.PSUM) as ps_out,
        ):
            # ---- PE clock pre-ramp: spin matmuls on a zero tile while the
            # ---- DMAs and the DVE setup below run. HAM releases the full
            # ---- clock only after ~5 us of sustained PE activity.
            dummy = constp.tile([128, T_PS], MM_DTYPE, tag="dummy")
            nc.gpsimd.memset(dummy[:], 0.0)
            spin_ps = ps_out.tile([128, 2 * T_PS], FP32, tag="o_ps", name="spin_ps")
            for s in range(N_SPIN):
                nc.tensor.matmul(
                    spin_ps[:, :T_PS], dummy[:, :128], dummy[:], start=True, stop=True
                )

            gw_t = constp.tile([G, M], MM_DTYPE, tag="gw_t")
            nc.gpsimd.dma_start(out=gw_t[:], in_=gwt_ext[:])

            # iota row [0..7] (cell centers); wy = relu(1 - |gp - j|)
            io_g = constp.tile([128, GS], FP32, tag="io_g")
            nc.gpsimd.iota(
                io_g[:],
                pattern=[[1, GS]],
                base=0,
                channel_multiplier=0,
                allow_small_or_imprecise_dtypes=True,
            )
            io = constp.tile([128, GS], FP32, tag="io")
            nc.vector.tensor_copy(io[:], io_g[:])

            # ---- ST build. All 6 (batch, chunk) column-pairs live in one
            # ---- [128, 12] tile, loaded with two rearranged DMAs.
            NCOL = 2 * BL * n_chunks
            pos_all = scr.tile([128, NCOL], FP32, tag="pos_all")
            nc.vector.memset(pos_all[:], 0.0)
            for b in range(BL):
                nc.gpsimd.dma_start(
                    out=pos_all[:, 4 * b : 4 * b + 4].rearrange(
                        "p (ci d) -> p ci d", ci=2
                    ),
                    in_=pos_ext[b, 0:256, :].rearrange("(ci p) d -> p ci d", p=128),
                )
                nc.gpsimd.dma_start(
                    out=pos_all[:50, 8 + 2 * b : 10 + 2 * b],
                    in_=pos_ext[b, 256:306, :],
                )
            # grid_pos = (pos + 1) * 4, exact vs reference's *8/2
            gp = scr.tile([128, NCOL], FP32, tag="gp")
            nc.vector.tensor_scalar(gp[:], pos_all[:], 1.0, GS / 2.0, OP.add, OP.mult)

            st_tiles = {}
            for b in range(BL):
                for ci, (c0, cn) in enumerate(C_CHUNKS):
                    sfx = f"{b}_{ci}"
                    col = _pos_col(b, ci)
                    wyx = []
                    for d, nm in ((0, "wy"), (1, "wx")):
                        cd = col + d
                        # hat(j) = max(0, min(1+(j-gp), 1-(j-gp)))
                        s1 = scr.tile([128, GS], FP32, tag=f"{nm}s{sfx}", name=f"{nm}s{sfx}")
                        nc.vector.tensor_scalar(
                            s1[:cn], io[:cn], gp[:cn, cd : cd + 1], 1.0,
                            OP.subtract, OP.add,
                        )
                        s2 = scr.tile([128, GS], FP32, tag=f"{nm}t{sfx}", name=f"{nm}t{sfx}")
                        nc.vector.tensor_scalar(
                            s2[:cn], s1[:cn], -1.0, 2.0, OP.mult, OP.add
                        )
                        mw = scr.tile([128, GS], FP32, tag=f"{nm}m{sfx}", name=f"{nm}m{sfx}")
                        nc.vector.tensor_tensor(mw[:cn], s1[:cn], s2[:cn], OP.min)
                        w = scr.tile([128, GS], FP32, tag=f"{nm}{sfx}", name=f"{nm}{sfx}")
                        nc.vector.tensor_single_scalar(w[:cn], mw[:cn], 0.0, OP.max)
                        wyx.append(w)
                    st = stp.tile([128, G], MM_DTYPE, tag=f"st{sfx}", name=f"st{sfx}")
                    nc.vector.tensor_tensor(
                        st[:cn].rearrange("c (i j) -> c i j", i=GS),
                        wyx[0][:cn].unsqueeze(2).broadcast_to((cn, GS, GS)),
                        wyx[1][:cn].unsqueeze(1).broadcast_to((cn, GS, GS)),
                        OP.mult,
                    )
                    st_tiles[(b, ci)] = st

            # Warm-up matmuls bridge the clock from spins into the main loop
            # and let the PE observe the DVE/DMA semaphores once.
            warm = ps_out.tile([128, 2 * T_PS], FP32, tag="o_ps", name="warm")
            for ci, (c0, cn) in enumerate(C_CHUNKS):
                st = st_tiles[(0, ci)]
                nc.tensor.matmul(
                    warm[:G, :G], st[:cn], st[:cn, :G], start=True, stop=True
                )
            nc.tensor.matmul(
                warm[:, :G], gw_t[:, :128], gw_t[:, :G], start=True, stop=True
            )

            # ---- Main loop: gv = ST.T @ x, out = GW @ gv, weights-outer.
            for b in range(BL):
                for tt in range(T // T_DMA):
                    t0 = tt * T_DMA
                    xts = []
                    for ci, (c0, cn) in enumerate(C_CHUNKS):
                        xt = xp.tile([128, T_DMA], MM_DTYPE, tag=f"x{ci}", name=f"x{ci}")
                        nc.gpsimd.dma_start(
                            out=xt[:cn], in_=x_ext[b, c0 : c0 + cn, t0 : t0 + T_DMA]
                        )
                        xts.append(xt)
                    outs = []
                    for mi, (m0, mn) in enumerate(M_CHUNKS):
                        outs.append(
                            outp.tile([128, T_DMA], MM_DTYPE, tag=f"o{mi}", name=f"o{mi}")
                        )

                    # mm1 for half h; gv copies (split in 512-halves across
                    # ACT and DVE) drain while the PE streams the next half.
                    gv_sbs = []
                    for h in range(NPS // 2):
                        gv_ps = ps_gv.tile(
                            [G, 2 * T_PS], FP32, tag=f"gv{h}", name=f"gv{h}"
                        )
                        for ci, (c0, cn) in enumerate(C_CHUNKS):
                            for q in range(2):
                                f0 = (h * 2 + q) * T_PS
                                nc.tensor.matmul(
                                    gv_ps[:, q * T_PS : (q + 1) * T_PS],
                                    st_tiles[(b, ci)][:cn],
                                    xts[ci][:cn, f0 : f0 + T_PS],
                                    start=(ci == 0),
                                    stop=(ci == n_chunks - 1),
                                    skip_group_check=True,
                                )
                        gv_sb = gvp.tile(
                            [G, 2 * T_PS], MM_DTYPE, tag=f"gv_sb{h}", name=f"gv_sb{h}"
                        )
                        nc.scalar.copy(gv_sb[:, :T_PS], gv_ps[:, :T_PS])
                        nc.vector.tensor_copy(gv_sb[:, T_PS:], gv_ps[:, T_PS:])
                        gv_sbs.append(gv_sb)

                    # mm2: h-outer so each half starts as soon as its gv lands
                    for h in range(NPS // 2):
                        for mi, (m0, mn) in enumerate(M_CHUNKS):
                            o_ps = ps_out.tile(
                                [128, 2 * T_PS], FP32, tag="o_ps", name="o_ps"
                            )
                            for q in range(2):
                                nc.tensor.matmul(
                                    o_ps[:mn, q * T_PS : (q + 1) * T_PS],
                                    gw_t[:, m0 : m0 + mn],
                                    gv_sbs[h][:, q * T_PS : (q + 1) * T_PS],
                                    start=True,
                                    stop=True,
                                )
                            f0 = h * 2 * T_PS
                            dst = outs[mi][:mn, f0 : f0 + 2 * T_PS]
                            if mi == 2 or (mi == 1 and h == 1):
                                nc.scalar.copy(dst, o_ps[:mn])
                            else:
                                nc.vector.tensor_copy(dst, o_ps[:mn])
                    for mi, (m0, mn) in enumerate(M_CHUNKS):
                        nc.sync.dma_start(
                            out=out_ext[b, m0 : m0 + mn, t0 : t0 + T_DMA],
                            in_=outs[mi][:mn],
                        )
    nc.compile()
    return nc


def make_in_maps(x, positions, grid_weights):
    gw_t = np.ascontiguousarray(grid_weights.T).astype(NP_MM)
    in_maps = []
    for i in range(N_CORES):
        sl = slice(i * BL, (i + 1) * BL)
        in_maps.append(
            {
                "x": np.ascontiguousarray(x[sl]).astype(NP_MM),
                "positions": np.ascontiguousarray(positions[sl]),
                "gw_t": gw_t,
            }
        )
    return in_maps


_NC_CACHE = None


def kernel(x, positions, grid_weights):
    global _NC_CACHE
    if _NC_CACHE is None:
        _NC_CACHE = build_nc()
    nc = _NC_CACHE
    in_maps = make_in_maps(x, positions, grid_weights)
    res = run_bass_kernel_spmd(nc, in_maps, core_ids=list(range(N_CORES)))
    out = np.concatenate([r["out"] for r in res.results], axis=0)
    return np.asarray(out, dtype=np.float32)


if __name__ == "__main__":
    xs = np.random.randn(B, C, T).astype(np.float32)
    ps = np.random.uniform(-1, 0.74, (B, C, 2)).astype(np.float32)
    gw = np.random.randn(M, G).astype(np.float32)
    out = kernel(xs, ps, gw)
    print(out.shape, out.dtype)


# revision 22
# speedup vs baseline: 1.2908x; 1.0054x over previous
"""AdaptiveGridMerger Trainium2 kernel.

Math: the reference scatters x[b,c,:] into a flat 8x8 grid with bilinear
(4-corner) weights from positions[b,c,:], then matmuls grid_weights.
Equivalent form used here: out[b] = GW @ (S[b] @ x[b]) where
S[b] in R[64,306] holds channel c's 4 corner weights in column c.
Row c of S[b].T factorizes as wy (x) wx with wy[j] = relu(1-|gp0-j|)
(the bilinear hat function; exact vs the reference's floor/frac math by
Sterbenz). Both contractions run on the TensorEngine in bf16.

Sharding: data-parallel over batch, 2 batches per core, grid_weights
replicated (pre-transposed on host to [64, 270] for the lhsT layout).

Perf structure: spin matmuls pre-ramp the PE clock (HAM gate) during
setup; weights-outer matmul ordering overlaps drain/fill; input DMAs
issue from GpSimd so output DMAs can't head-of-line block them; PSUM ->
SBUF copies are split across DVE and ACT so the PE never stalls.
"""

import numpy as np

import concourse.bass as bass
import concourse.bacc as bacc
import concourse.mybir as mybir
from concourse import tile
from concourse.bass_utils import run_bass_kernel_spmd

B, C, T = 16, 306, 4096
M, G, GS = 270, 64, 8
N_CORES = 8
BL = B // N_CORES  # batches per core

C_CHUNKS = [(0, 128), (128, 128), (256, 50)]
M_CHUNKS = [(0, 128), (128, 128), (256, 14)]
T_DMA = 2048
T_PS = 512
NPS = T_DMA // T_PS
N_SPIN = 14

MM_DTYPE = mybir.dt.bfloat16
NP_MM = mybir.dt.np(MM_DTYPE)

FP32 = mybir.dt.float32
OP = mybir.AluOpType


def _pos_col(b, ci):
    # column pair layout in pos_all/gp: full 128-row chunks first (one
    # rearranged DMA), the two 50-row tails last (second DMA)
    return 2 * (b * 2 + ci) if ci < 2 else 8 + 2 * b


def build_nc():
    nc = bacc.Bacc()
    x_ext = nc.declare_dram_parameter("x", [BL, C, T], MM_DTYPE, isOutput=False)
    pos_ext = nc.declare_dram_parameter("positions", [BL, C, 2], FP32, isOutput=False)
    gwt_ext = nc.declare_dram_parameter("gw_t", [G, M], MM_DTYPE, isOutput=False)
    out_ext = nc.declare_dram_parameter("out", [BL, M, T], MM_DTYPE, isOutput=True)

    n_chunks = len(C_CHUNKS)
    with tile.TileContext(nc) as tc:
        with (
            tc.tile_pool(name="const", bufs=1) as constp,
            tc.tile_pool(name="stp", bufs=1) as stp,
            tc.tile_pool(name="scr", bufs=1) as scr,
            tc.tile_pool(name="xp", bufs=1) as xp,
            tc.tile_pool(name="op", bufs=2) as outp,
            tc.tile_pool(name="gvp", bufs=3) as gvp,
            tc.tile_pool(name="ps_gv", bufs=1, space=bass.MemorySpace.PSUM) as ps_gv,
            tc.tile_pool(name="ps_out", bufs=2, space=bass.MemorySpace.PSUM) as ps_out,
        ):
            # ---- PE clock pre-ramp: spin matmuls on a zero tile while the
            # ---- DMAs and the DVE setup below run. HAM releases the full
            # ---- clock only after ~5 us of sustained PE activity.
            dummy = constp.tile([128, T_PS], MM_DTYPE, tag="dummy")
            nc.gpsimd.memset(dummy[:], 0.0)
            spin_ps = ps_out.tile([128, 2 * T_PS], FP32, tag="o_ps", name="spin_ps")
            for s in range(N_SPIN):
                nc.tensor.matmul(
                    spin_ps[:, :T_PS], dummy[:, :128], dummy[:], start=True, stop=True
                )

            gw_t = constp.tile([G, M], MM_DTYPE, tag="gw_t")
            nc.sync.dma_start(out=gw_t[:], in_=gwt_ext[:])

            # iota row [0..7] (cell centers); wy = relu(1 - |gp - j|)
            io_g = constp.tile([128, GS], FP32, tag="io_g")
            nc.gpsimd.iota(
                io_g[:],
                pattern=[[1, GS]],
                base=0,
                channel_multiplier=0,
                allow_small_or_imprecise_dtypes=True,
            )
            io = constp.tile([128, GS], FP32, tag="io")
            nc.vector.tensor_copy(io[:], io_g[:])

            # ---- ST build. All 6 (batch, chunk) column-pairs live in one
            # ---- [128, 12] tile, loaded with two rearranged DMAs.
            NCOL = 2 * BL * n_chunks
            pos_all = scr.tile([128, NCOL], FP32, tag="pos_all")
            nc.vector.memset(pos_all[:], 0.0)
            for b in range(BL):
                nc.sync.dma_start(
                    out=pos_all[:, 4 * b : 4 * b + 4].rearrange(
                        "p (ci d) -> p ci d", ci=2
                    ),
                    in_=pos_ext[b, 0:256, :].rearrange("(ci p) d -> p ci d", p=128),
                )
                nc.sync.dma_start(
                    out=pos_all[:50, 8 + 2 * b : 10 + 2 * b],
                    in_=pos_ext[b, 256:306, :],
                )
            # prefetch ALL x tiles up front: the issues run before any
            # out-DMA can head-of-line block them, and transfers stream in
            # the background across the HW queues.
            x_tiles = {}
            for b in range(BL):
                for tt in range(T // T_DMA):
                    t0 = tt * T_DMA
                    for ci, (c0, cn) in enumerate(C_CHUNKS):
                        xt = xp.tile(
                            [128, T_DMA], MM_DTYPE,
                            tag=f"x{b}_{tt}_{ci}", name=f"x{b}_{tt}_{ci}",
                        )
                        nc.sync.dma_start(
                            out=xt[:cn], in_=x_ext[b, c0 : c0 + cn, t0 : t0 + T_DMA]
                        )
                        x_tiles[(b, tt, ci)] = xt
            # grid_pos = (pos + 1) * 4, exact vs reference's *8/2
            gp = scr.tile([128, NCOL], FP32, tag="gp")
            nc.vector.tensor_scalar(gp[:], pos_all[:], 1.0, GS / 2.0, OP.add, OP.mult)

            st_tiles = {}
            for b in range(BL):
                for ci, (c0, cn) in enumerate(C_CHUNKS):
                    sfx = f"{b}_{ci}"
                    col = _pos_col(b, ci)
                    wyx = []
                    for d, nm in ((0, "wy"), (1, "wx")):
                        cd = col + d
                        # hat(j) = max(0, min(1+(j-gp), 1-(j-gp)))
                        s1 = scr.tile([128, GS], FP32, tag=f"{nm}s{sfx}", name=f"{nm}s{sfx}")
                        nc.vector.tensor_scalar(
                            s1[:cn], io[:cn], gp[:cn, cd : cd + 1], 1.0,
                            OP.subtract, OP.add,
                        )
                        s2 = scr.tile([128, GS], FP32, tag=f"{nm}t{sfx}", name=f"{nm}t{sfx}")
                        nc.vector.tensor_scalar(
                            s2[:cn], s1[:cn], -1.0, 2.0, OP.mult, OP.add
                        )
                        mw = scr.tile([128, GS], FP32, tag=f"{nm}m{sfx}", name=f"{nm}m{sfx}")
                        nc.vector.tensor_tensor(mw[:cn], s1[:cn], s2[:cn], OP.min)
                        w = scr.tile([128, GS], FP32, tag=f"{nm}{sfx}", name=f"{nm}{sfx}")
                        nc.vector.tensor_single_scalar(w[:cn], mw[:cn], 0.0, OP.max)
                        wyx.append(w)
                    st = stp.tile([128, G], MM_DTYPE, tag=f"st{sfx}", name=f"st{sfx}")
                    nc.vector.tensor_tensor(
                        st[:cn].rearrange("c (i j) -> c i j", i=GS),
                        wyx[0][:cn].unsqueeze(2).broadcast_to((cn, GS, GS)),
                        wyx[1][:cn].unsqueeze(1).broadcast_to((cn, GS, GS)),
                        OP.mult,
                    )
                    st_tiles[(b, ci)] = st

            # Warm-up matmuls bridge the clock from spins into the main loop
            # and let the PE observe the DVE/DMA semaphores once.
            warm = ps_out.tile([128, 2 * T_PS], FP32, tag="o_ps", name="warm")
            for ci, (c0, cn) in enumerate(C_CHUNKS):
                st = st_tiles[(0, ci)]
                nc.tensor.matmul(
                    warm[:G, :G], st[:cn], st[:cn, :G], start=True, stop=True
                )
            nc.tensor.matmul(
                warm[:, :G], gw_t[:, :128], gw_t[:, :G], start=True, stop=True
            )

            # ---- Main loop: gv = ST.T @ x, out = GW @ gv, weights-outer.
            for b in range(BL):
                for tt in range(T // T_DMA):
                    t0 = tt * T_DMA
                    xts = [x_tiles[(b, tt, ci)] for ci in range(n_chunks)]
                    outs = []
                    for mi, (m0, mn) in enumerate(M_CHUNKS):
                        outs.append(
                            outp.tile([128, T_DMA], MM_DTYPE, tag=f"o{mi}", name=f"o{mi}")
                        )

                    # mm1 for half h; gv copies (split in 512-halves across
                    # ACT and DVE) drain while the PE streams the next half.
                    gv_sbs = []
                    for h in range(NPS // 2):
                        gv_ps = ps_gv.tile(
                            [G, 2 * T_PS], FP32, tag=f"gv{h}", name=f"gv{h}"
                        )
                        for ci, (c0, cn) in enumerate(C_CHUNKS):
                            for q in range(2):
                                f0 = (h * 2 + q) * T_PS
                                nc.tensor.matmul(
                                    gv_ps[:, q * T_PS : (q + 1) * T_PS],
                                    st_tiles[(b, ci)][:cn],
                                    xts[ci][:cn, f0 : f0 + T_PS],
                                    start=(ci == 0),
                                    stop=(ci == n_chunks - 1),
                                    skip_group_check=True,
                                )
                        gv_sb = gvp.tile(
                            [G, 2 * T_PS], MM_DTYPE, tag=f"gv_sb{h}", name=f"gv_sb{h}"
                        )
                        if h == 0:
                            nc.scalar.copy(gv_sb[:, :T_PS], gv_ps[:, :T_PS])
                            nc.scalar.copy(gv_sb[:, T_PS:], gv_ps[:, T_PS:])
                        else:
                            nc.vector.tensor_copy(gv_sb[:], gv_ps[:])
                        gv_sbs.append(gv_sb)

                    # mm2: h-outer so each half starts as soon as its gv lands
                    for h in range(NPS // 2):
                        for mi, (m0, mn) in enumerate(M_CHUNKS):
                            o_ps = ps_out.tile(
                                [128, 2 * T_PS], FP32, tag="o_ps", name="o_ps"
                            )
                            for q in range(2):
                                nc.tensor.matmul(
                                    o_ps[:mn, q * T_PS : (q + 1) * T_PS],
                                    gw_t[:, m0 : m0 + mn],
                                    gv_sbs[h][:, q * T_PS : (q + 1) * T_PS],
                                    start=True,
                                    stop=True,
                                )
                            f0 = h * 2 * T_PS
                            dst = outs[mi][:mn, f0 : f0 + 2 * T_PS]
                            if mi == 2 or (mi == 1 and h == 1):
                                nc.scalar.copy(dst, o_ps[:mn])
                            else:
                                nc.vector.tensor_copy(dst, o_ps[:mn])
                    for mi, (m0, mn) in enumerate(M_CHUNKS):
                        nc.sync.dma_start(
                            out=out_ext[b, m0 : m0 + mn, t0 : t0 + T_DMA],
                            in_=outs[mi][:mn],
                        )
    nc.compile()
    return nc


def make_in_maps(x, positions, grid_weights):
    gw_t = np.ascontiguousarray(grid_weights.T).astype(NP_MM)
    in_maps = []
    for i in range(N_CORES):
        sl = slice(i * BL, (i + 1) * BL)
        in_maps.append(
            {
                "x": np.ascontiguousarray(x[sl]).astype(NP_MM),
                "positions": np.ascontiguousarray(positions[sl]),
                "gw_t": gw_t,
            }
        )
    return in_maps


_NC_CACHE = None


def kernel(x, positions, grid_weights):
    global _NC_CACHE
    if _NC_CACHE is None:
        _NC_CACHE = build_nc()
    nc = _NC_CACHE
    in_maps = make_in_maps(x, positions, grid_weights)
    res = run_bass_kernel_spmd(nc, in_maps, core_ids=list(range(N_CORES)))
    out = np.concatenate([r["out"] for r in res.results], axis=0)
    return np.asarray(out, dtype=np.float32)


if __name__ == "__main__":
    xs = np.random.randn(B, C, T).astype(np.float32)
    ps = np.random.uniform(-1, 0.74, (B, C, 2)).astype(np.float32)
    gw = np.random.randn(M, G).astype(np.float32)
    out = kernel(xs, ps, gw)
    print(out.shape, out.dtype)
